# revision 1
# baseline (speedup 1.0000x reference)
"""Causal multi-head attention (B=2, S=2048, E=1024, H=16, D=64) on 8 trn2 NeuronCores.

Sharding: core c handles batch b = c // 4 and head group g = c % 4 (4 heads each).
Each core computes, for its batch and its 4 heads:
    q/k/v = x @ W[qkv][:, 256g:256g+256], causal attention, then the partial
    projection  out_heads @ Wp[256g:256g+256, :]  -> [2048, 1024].
Host gathers: out[b] = sum_g partial[b, g] + bp  (the "all-reduce" of the TP hint).

On-core layout is fully transposed: xT [E, S] is built with PE transposes; qT/kT
are produced directly as [head_dim, S]; scores are computed transposed
(sT[j, q] = k_j . q_q) so softmax denominators come from an extra ones-column in
v (fused into the PV matmul, M=65), and attention output oT [hd, S] feeds the
final projection as its stationary operand without any further transposes.
Matmul operands are float32r (single-pass fp32, 4x faster than full fp32 on the
PE); set MM_DT = float32 for full precision at ~4x the matmul cost.
"""

import os
import sys
import numpy as np

sys.path.insert(0, "/opt/trn_rl_repo")

import concourse.bass as bass
import concourse.bacc as bacc_mod
import concourse.mybir as mybir
import concourse.tile as tile
from concourse.masks import make_identity
from concourse import library_config

F32 = mybir.dt.float32
F32R = mybir.dt.float32r
P = 128

B = 2
S = 2048
E = 1024
NHEADS_TOTAL = 16
D = 64
N_CORES = 8
GROUPS = 4            # head groups (tensor parallel)
HD = NHEADS_TOTAL * D // GROUPS   # 256 head-dims per core
QB = 512              # q-block width

MM_DT = F32R          # matmul dtype (float32r: 1 cyc/row, ~tf32 precision)


def build_core_program(mm_dt=MM_DT, lower_isa=True):
    """One NeuronCore's program (SPMD: all 8 cores run this on different data)."""
    nc = bacc_mod.Bacc()
    x_d = nc.declare_dram_parameter("x", [S, E], F32, False)
    wq_d = nc.declare_dram_parameter("wq", [E, HD], F32, False)
    wk_d = nc.declare_dram_parameter("wk", [E, HD], F32, False)
    wv_d = nc.declare_dram_parameter("wv", [E, HD], F32, False)
    wp_d = nc.declare_dram_parameter("wp", [HD, E], F32, False)
    y_d = nc.declare_dram_parameter("y", [S, E], F32, True)

    NH = HD // D          # heads per core (4)
    NHP = HD // P         # head pairs (2)
    NST = S // P          # s tiles (16)
    NEC = E // P          # e chunks (8)
    NQB = S // QB         # q blocks (4)
    JPQ = QB // P         # j tiles per q block (4)
    VW = D + 1            # 65: v columns + ones column
    MDT = mm_dt

    with tile.TileContext(nc) as tc:
        nc.gpsimd.load_library(library_config.attn)
        from contextlib import ExitStack
        with ExitStack() as ctx:
            const = ctx.enter_context(tc.tile_pool(name="const", bufs=1))
            persist = ctx.enter_context(tc.tile_pool(name="persist", bufs=1))

            # f32r tiles can't be memset directly (no f32r memset ISA);
            # build f32 staging data and round-copy into the f32r tiles.
            ident_f32 = const.tile([P, P], F32)
            make_identity(nc, ident_f32)
            ident = const.tile([P, P], MDT)
            nc.vector.tensor_copy(ident[:], ident_f32[:])

            qT = [persist.tile([P, S], MDT, tag=f"qT{hp}", name=f"qT{hp}")
                  for hp in range(NHP)]
            kT = [persist.tile([P, S], MDT, tag=f"kT{hp}", name=f"kT{hp}")
                  for hp in range(NHP)]
            v_ext = persist.tile([P, NST * NH * VW], MDT, tag="v_ext", name="v_ext")
            # ones-columns (index 64 of each head slab); the v copies below
            # only overwrite columns 0:63 of each slab.
            ones_f32 = const.tile([P, NST * NH], F32)
            nc.vector.memset(ones_f32[:], 1.0)
            nc.vector.tensor_copy(
                v_ext.rearrange("p (s c) -> p s c", c=VW)[:, :, D:VW],
                ones_f32.rearrange("p (s o) -> p s o", o=1),
            )
            oT_all = [persist.tile([P, S], MDT, tag=f"oT{hp}", name=f"oT{hp}")
                      for hp in range(NHP)]
            xT = [persist.tile([P, S], MDT, tag=f"xT{ec}", name=f"xT{ec}")
                  for ec in range(NEC)]
            wv_sb = persist.tile([P, NEC * HD], MDT, tag="wv", name="wv_sb")

            # ---------------- Stage A+B: xT, then q/k/v ----------------
            with tc.tile_pool(name="wqkv", bufs=1) as wpool, \
                 tc.tile_pool(name="xnat", bufs=8) as xnat_pool, \
                 tc.tile_pool(name="tp_ps", bufs=3, space="PSUM") as tp_ps, \
                 tc.tile_pool(name="qk_ps", bufs=3, space="PSUM") as qk_ps, \
                 tc.tile_pool(name="v_ps", bufs=2, space="PSUM") as v_ps:

                def load_chunk(sc):
                    lst = []
                    for k in range(4):
                        st = 4 * sc + k
                        t = xnat_pool.tile([P, E], MDT, tag="xn", name="xn")
                        nc.sync.dma_start(
                            out=t, in_=x_d[P * st:P * (st + 1), :].bitcast(MDT))
                        lst.append(t)
                    return lst

                # x chunk 0 first (PE transposes can start ASAP), then weights
                xn_cur = load_chunk(0)
                wsb = {"wv": wv_sb}
                for nm, wd in (("wq", wq_d), ("wk", wk_d)):
                    t = wpool.tile([P, NEC * HD], MDT, tag=nm, name=f"{nm}_sb")
                    wsb[nm] = t
                for nm, wd in (("wq", wq_d), ("wk", wk_d), ("wv", wv_d)):
                    nc.sync.dma_start(
                        out=wsb[nm].rearrange("p (c n) -> p c n", c=NEC),
                        in_=wd[:, :].bitcast(MDT).rearrange("(c p) n -> p c n", p=P),
                    )

                for sc in range(NQB):          # 512-wide s-chunk
                    xn = xn_cur
                    if sc + 1 < NQB:
                        xn_cur = load_chunk(sc + 1)   # prefetch next chunk
                    # transpose this s-chunk into all 8 xT e-chunks
                    for ec in range(NEC):
                        pt = tp_ps.tile([P, 512], MDT, tag="pt", name="pt")
                        for k in range(4):
                            nc.tensor.transpose(
                                pt[:, 128 * k:128 * (k + 1)],
                                xn[k][:, P * ec:P * (ec + 1)],
                                ident,
                            )
                        nc.any.tensor_copy(xT[ec][:, QB * sc:QB * (sc + 1)], pt[:])
                    # qT / kT for this s-chunk
                    for nm, dest in (("wq", qT), ("wk", kT)):
                        for hp in range(NHP):
                            ps = qk_ps.tile([P, 512], F32, tag="qk", name="qk")
                            for ec in range(NEC):
                                nc.tensor.matmul(
                                    ps[:],
                                    wsb[nm][:, ec * HD + P * hp:ec * HD + P * (hp + 1)],
                                    xT[ec][:, QB * sc:QB * (sc + 1)],
                                    start=(ec == 0), stop=(ec == NEC - 1),
                                )
                            nc.any.tensor_copy(dest[hp][:, QB * sc:QB * (sc + 1)], ps[:])
                    # v (natural layout): first half inline, rest deferred
                    # into the attention loop as PE filler work
                    for k in range(4):
                        st = 4 * sc + k
                        if st >= NST // 2:
                            continue
                        vp = v_ps.tile([P, HD], F32, tag="vp", name="vp")
                        for ec in range(NEC):
                            nc.tensor.matmul(
                                vp[:],
                                xT[ec][:, P * st:P * (st + 1)],
                                wsb["wv"][:, ec * HD:(ec + 1) * HD],
                                start=(ec == 0), stop=(ec == NEC - 1),
                            )
                        vslab = v_ext[:, NH * VW * st:NH * VW * (st + 1)]
                        nc.any.tensor_copy(
                            vslab.rearrange("p (h c) -> p h c", h=NH)[:, :, 0:D],
                            vp.rearrange("p (h c) -> p h c", h=NH),
                        )

            # ---------- Stage C+D: attention + projection, per q-block ----------
            with tc.tile_pool(name="pT", bufs=4) as pT_pool, \
                 tc.tile_pool(name="dnm", bufs=6) as dnm_pool, \
                 tc.tile_pool(name="rbp", bufs=4) as rb_pool, \
                 tc.tile_pool(name="wp_sb", bufs=1) as wp_pool, \
                 tc.tile_pool(name="ysb", bufs=2) as y_pool, \
                 tc.tile_pool(name="sT_ps", bufs=3, space="PSUM") as sT_ps, \
                 tc.tile_pool(name="oT_ps", bufs=2, space="PSUM") as oT_ps:

                wp_sb = [wp_pool.tile([P, E], MDT, tag=f"wp{hp}", name=f"wp{hp}")
                         for hp in range(NHP)]
                for hp in range(NHP):
                    nc.sync.dma_start(
                        out=wp_sb[hp],
                        in_=wp_d[P * hp:P * (hp + 1), :].bitcast(MDT))

                # deferred PE filler jobs, interleaved into the attention
                # loop so the PE has dependency-free work between QK/PV
                # bursts: first the second half of the v tiles, then the
                # projection of each finished q-block.
                filler_jobs = [("v", st) for st in range(NST // 2, NST)]

                def emit_v(st):
                    vp = sT_ps.tile([P, HD], F32, tag="sT", name="vp")
                    for ec in range(NEC):
                        nc.tensor.matmul(
                            vp[:],
                            xT[ec][:, P * st:P * (st + 1)],
                            wv_sb[:, ec * HD:(ec + 1) * HD],
                            start=(ec == 0), stop=(ec == NEC - 1),
                        )
                    vslab = v_ext[:, NH * VW * st:NH * VW * (st + 1)]
                    nc.vector.tensor_copy(
                        vslab.rearrange("p (h c) -> p h c", h=NH)[:, :, 0:D],
                        vp.rearrange("p (h c) -> p h c", h=NH),
                    )

                def emit_proj(qt):
                    ysb = y_pool.tile([P, E], F32, tag="ysb", name="ysb")
                    for nk in range(E // 512):
                        pj = sT_ps.tile([P, 512], F32, tag="sT", name="pj")
                        for hp in range(NHP):
                            nc.tensor.matmul(
                                pj[:],
                                oT_all[hp][:, P * qt:P * (qt + 1)],
                                wp_sb[hp][:, 512 * nk:512 * (nk + 1)],
                                start=(hp == 0), stop=(hp == NHP - 1),
                            )
                        nc.vector.tensor_copy(ysb[:, 512 * nk:512 * (nk + 1)],
                                              pj[:])
                    nc.sync.dma_start(out=y_d[P * qt:P * (qt + 1), :], in_=ysb)

                filler_tick = [0]

                def emit_one_proj():
                    # spread fillers over every other group so they last
                    # through the long final q-block instead of bunching up
                    filler_tick[0] += 1
                    if filler_tick[0] % 2 or not filler_jobs:
                        return
                    kind, arg = filler_jobs.pop(0)
                    if kind == "v":
                        emit_v(arg)
                    else:
                        emit_proj(arg)

                for qb in range(NQB):
                    for hp in range(NHP):
                        n_j = (qb + 1) * JPQ
                        oT2 = [oT_ps.tile([P, 512], F32, tag="oT", name="oT")
                               for _ in range(2)]
                        for jg in range(n_j // 2):
                            sT2 = [sT_ps.tile([P, 1024], F32, tag="sT", name="sT")
                                   for _ in range(2)]
                            pT2 = [pT_pool.tile([P, 1024], MDT, tag="pT", name="pT")
                                   for _ in range(2)]
                            # c0[t]: first q-column this j-tile can see
                            # (causal); matmul N stays >= 256 (f32r small-N
                            # penalty), so the streamed region is [c0mm, 512).
                            c0m, c0s = [], []
                            for t in range(2):
                                js = 2 * jg + t
                                cm = max(0, P * js - QB * qb)   # true diagonal col
                                c0m.append(cm)
                                c0s.append(min(cm, 512 - 256))  # streamed from here
                            for t in range(2):
                                js = 2 * jg + t
                                for h in range(2):  # head within pair
                                    lo, hi = D * h, D * (h + 1)
                                    nc.tensor.matmul(
                                        sT2[h][:, 512 * t:512 * (t + 1)],
                                        kT[hp][lo:hi, P * js:P * (js + 1)],
                                        qT[hp][lo:hi, QB * qb:QB * (qb + 1)],
                                        start=True, stop=True,
                                        tile_position=(lo, 0),
                                    )
                            ce0 = c0s[0]   # masked prefix of the group's
                            for h in range(2):  # first tile is never read
                                nc.scalar.activation(
                                    pT2[h][:, ce0:1024], sT2[h][:, ce0:1024],
                                    mybir.ActivationFunctionType.Exp, scale=0.125)
                            for t in range(2):
                                js = 2 * jg + t
                                if js >= JPQ * qb:   # diagonal tile: causal mask
                                    cs, cm = c0s[t], c0m[t]
                                    ce = min(cm + P, 512)  # triangle ends here
                                    w = ce - cs
                                    for h in range(2):
                                        nc.gpsimd.affine_select(
                                            out=pT2[h][:, 512 * t + cs:
                                                       512 * t + ce],
                                            in_=pT2[h][:, 512 * t + cs:
                                                       512 * t + ce],
                                            pattern=[[1, w]],
                                            compare_op=mybir.AluOpType.is_ge,
                                            fill=0.0,
                                            base=QB * qb - P * js + cs,
                                            channel_multiplier=-1,
                                        )
                            for t in range(2):
                                js = 2 * jg + t
                                cs = c0s[t]
                                for h in range(2):
                                    hl = 2 * hp + h  # head index within core
                                    nc.tensor.matmul(
                                        oT2[h][0:VW, cs:512],
                                        v_ext[:, NH * VW * js + VW * hl:
                                              NH * VW * js + VW * (hl + 1)],
                                        pT2[h][:, 512 * t + cs:512 * (t + 1)],
                                        start=(js == 0), stop=(js == n_j - 1),
                                    )
                            emit_one_proj()
                        # normalize by the ones-column row (row 64 = sum of
                        # exp). Copy numerator + denominator out of PSUM right
                        # away so the oT bank frees for the next section; the
                        # recip/broadcast/mul chain then runs off-path.
                        # (reciprocal_approx_fast / partition_broadcast need
                        # inputs at partition base 0.)
                        for h in range(2):
                            onum = rb_pool.tile([D, 512], F32, tag="onum",
                                                name="onum", bufs=2)
                            nc.vector.tensor_copy(onum[:], oT2[h][0:D, :])
                            dn = dnm_pool.tile([1, 512], F32, tag="dn", name="dn")
                            nc.vector.tensor_copy(dn[0:1, :], oT2[h][D:VW, :])
                            dr = dnm_pool.tile([1, 512], F32, tag="dr", name="dr")
                            nc.vector.reciprocal_approx_fast(dr[0:1, :], dn[0:1, :])
                            rb = rb_pool.tile([D, 512], F32, tag="rb", name="rb")
                            nc.gpsimd.partition_broadcast(
                                rb[:], dr[0:1, :], channels=D)
                            nc.vector.tensor_mul(
                                oT_all[hp][D * h:D * (h + 1), QB * qb:QB * (qb + 1)],
                                onum[:], rb[:])

                    filler_jobs.extend(
                        ("proj", qt) for qt in range(JPQ * qb, JPQ * (qb + 1)))
                while filler_jobs:
                    emit_one_proj()

    if lower_isa:
        # Bacc.finalize -> compile(): wait splitting (1 wait/inst on TRN2),
        # matmul-wait hoisting to ldweights, library/act-table load insertion,
        # ISA byte codegen. Without this walrus rejects the module.
        nc.finalize()
    return nc


_CACHED_NC = None


def _get_nc():
    global _CACHED_NC
    if _CACHED_NC is None:
        _CACHED_NC = build_core_program()
    return _CACHED_NC


def shard_inputs(x, Wq, Wk, Wv, Wp):
    in_maps = []
    for core in range(N_CORES):
        b, g = core // GROUPS, core % GROUPS
        sl = slice(HD * g, HD * (g + 1))
        in_maps.append({
            "x": np.ascontiguousarray(x[b], dtype=np.float32),
            "wq": np.ascontiguousarray(Wq[:, sl], dtype=np.float32),
            "wk": np.ascontiguousarray(Wk[:, sl], dtype=np.float32),
            "wv": np.ascontiguousarray(Wv[:, sl], dtype=np.float32),
            "wp": np.ascontiguousarray(Wp[sl, :], dtype=np.float32),
        })
    return in_maps


def _ensure_ntff_hook():
    """Provide antenv.axon_hooks (missing in this image) so trace=True can
    collect NTFF profiles through libaxon_pjrt's nrt-profile C ABI."""
    import types
    try:
        from antenv.axon_hooks import get_axon_ntff_profile_hook  # noqa: F401
        return
    except ImportError:
        pass
    import antenv
    mod = types.ModuleType("antenv.axon_hooks")
    mod._hook = None
    def set_axon_ntff_profile_hook(h):
        mod._hook = h
    def get_axon_ntff_profile_hook():
        return mod._hook
    mod.set_axon_ntff_profile_hook = set_axon_ntff_profile_hook
    mod.get_axon_ntff_profile_hook = get_axon_ntff_profile_hook
    sys.modules["antenv.axon_hooks"] = mod
    antenv.axon_hooks = mod
    try:
        from trn_agent_boot.trn_boot import _ntff_profile_via_ctypes
        mod._hook = _ntff_profile_via_ctypes("/opt/axon/libaxon_pjrt.so")
    except Exception as e:  # degrade: tracing skipped, run still works
        print(f"ntff hook setup failed: {e}", file=sys.stderr)


def run(inputs, trace=False, **spmd_kwargs):
    """Returns (full_output [B,S,E], BassKernelResults)."""
    from concourse.bass_utils import run_bass_kernel_spmd
    if trace:
        _ensure_ntff_hook()
    x = np.asarray(inputs["x"], dtype=np.float32)
    Wq = np.asarray(inputs["Wq"], dtype=np.float32)
    Wk = np.asarray(inputs["Wk"], dtype=np.float32)
    Wv = np.asarray(inputs["Wv"], dtype=np.float32)
    Wp = np.asarray(inputs["Wp"], dtype=np.float32)
    bp = np.asarray(inputs["bp"], dtype=np.float32)

    nc = _get_nc()
    in_maps = shard_inputs(x, Wq, Wk, Wv, Wp)
    res = run_bass_kernel_spmd(nc, in_maps, list(range(N_CORES)),
                               trace=trace, **spmd_kwargs)
    out = np.zeros((B, S, E), dtype=np.float32)
    for core in range(N_CORES):
        out[core // GROUPS] += res.results[core]["y"]
    out += bp[None, None, :]
    return out, res


def kernel(x, Wq, Wk, Wv, Wp, bp):
    out, _ = run({"x": x, "Wq": Wq, "Wk": Wk, "Wv": Wv, "Wp": Wp, "bp": bp})
    return out

